# revision 26
# baseline (speedup 1.0000x reference)
"""Trainium2 Bass kernel for nn_Correct_PrototypeManager (segment_reduce).

Reference computation:
    pred_lbl = argmax(preds, axis=1)                      # [B, H, W]
    feats_up = bilinear_resize(feats, H, W)               # [B, C, H, W]
    joint[b,k,h,w] = (masks==k) & (pred_lbl==k)
    counts[b,k] = sum_hw joint ; sums[b,k,c] = sum_hw feats_up * joint
    proto = mean_b( sums / (counts + eps) )               # [K, C]

Algebraic transform: bilinear upsample is linear, feats_up = (Uh (x) Uw) @
feats, so sums[k,c] = <U^T joint_k, feats_c>: downsample the one-hot joint
map (256^2 -> 64^2) with the adjoint of the upsample and contract over 4096
coarse pixels. Counts are preserved exactly (rows of U sum to 1).

Design (vs the 124us f32 baseline; see git-less history in kernel_v*.py):
  - preds stays f32: the argmax must match f32 exactly (one flipped pixel
    in a ~140-pixel class mean already exceeds the 2e-2 gate). argmax via
    a tree of f32 TT-maxes; eq = is_ge(preds, max) -> fp16.
  - everything downstream is fp16 and EXACT: U weights are multiples of
    1/8, A <= ~4 and ds <= ~16.25 stay on representable grids; PE
    accumulates in f32 PSUM. Only feats fp16 rounding (~5e-5 rel) leaks.
  - one-hot of mask via 21 tensor_scalar is_equal ops (4x DVE mode).
  - all host->device tensors are laid out so every DMA is 128 contiguous
    per-partition runs (128 descriptors, line-rate); preds halves arrive
    in two k-chunks chased by the max-tree.
  - stage 1 (Uh^T joint) is computed per hf-half into separate A0/A1 so
    the whole h0 chain (matmuls, transposes, copies) overlaps the DVE's
    h1 work; stage 2 accumulates all four (half, wf-half) contributions
    in PSUM via a [U|U] stationary putting ds on all 128 partitions.
  - the final matmul uses ds chunks as stationary and feats^T [+ a ones
    column] as moving, so counts fall out as output column C for free.

Sharding: data-parallel over batch B=8, one image per NeuronCore; the
[K, C+1] per-image partials (sums | counts) are combined on host.
"""

import numpy as np

B = 8
C = 256
K = 21
HC = WC = 64
HF = WF = 256
EPS = 1e-6
N_CORES = 8
PIX = HC * WC  # 4096
KW = K * WF    # 5376
KA = 11        # preds k-chunk split: classes [0,KA) then [KA,K)

_PROGRAM_CACHE: dict = {}


def _upsample_matrix(n_in: int, n_out: int) -> np.ndarray:
    """U [n_out, n_in] with resize(x, 'bilinear', half-pixel) == U @ x."""
    U = np.zeros((n_out, n_in), dtype=np.float64)
    scale = n_in / n_out
    for i in range(n_out):
        src = (i + 0.5) * scale - 0.5
        f = int(np.floor(src))
        w = src - f
        lo = min(max(f, 0), n_in - 1)
        hi = min(max(f + 1, 0), n_in - 1)
        U[i, lo] += 1.0 - w
        U[i, hi] += w
    return U.astype(np.float32)


def _build_program():
    import concourse.bass as bass
    import concourse.bacc as bacc
    import concourse.tile as tile
    from concourse import mybir
    from contextlib import ExitStack

    f16 = mybir.dt.float16
    f32 = mybir.dt.float32
    OP = mybir.AluOpType

    nc = bacc.Bacc("TRN2", target_bir_lowering=False, debug=False,
                   num_devices=N_CORES)

    preds_d = nc.dram_tensor("preds", [2, 128, K, WF], f32,
                             kind="ExternalInput")
    mask_d = nc.dram_tensor("mask", [128, 2, WF], f16, kind="ExternalInput")
    feats_d = nc.dram_tensor("feats", [128, 32, C + 1], f16,
                             kind="ExternalInput")
    u_d = nc.dram_tensor("u", [2, 128, HC], f16, kind="ExternalInput")
    ucat_d = nc.dram_tensor("ucat", [2, 128, 128], f16, kind="ExternalInput")
    ident_d = nc.dram_tensor("ident", [64, 64], f16, kind="ExternalInput")
    out_d = nc.dram_tensor("out", [K, C + 1], f32, kind="ExternalOutput")

    with tile.TileContext(nc) as tc, ExitStack() as ctx:
        const_pool = ctx.enter_context(tc.tile_pool(name="const", bufs=1))
        data_pool = ctx.enter_context(tc.tile_pool(name="data", bufs=1))
        res_pool = ctx.enter_context(tc.tile_pool(name="res", bufs=1))
        ps1_pool = ctx.enter_context(
            tc.tile_pool(name="ps1", bufs=2, space="PSUM"))
        pst_pool = ctx.enter_context(
            tc.tile_pool(name="pst", bufs=1, space="PSUM"))
        psb_pool = ctx.enter_context(
            tc.tile_pool(name="psb", bufs=4, space="PSUM"))
        psf_pool = ctx.enter_context(
            tc.tile_pool(name="psf", bufs=1, space="PSUM"))

        # ---- constants (scalar/ACT DMA ring: tiny, instant) ----
        u16_t = []
        ucat_t = []
        for h in range(2):
            t = const_pool.tile([128, HC], f16, tag=f"u16_{h}")
            nc.scalar.dma_start(t[:], u_d.ap()[h])
            u16_t.append(t)
            t = const_pool.tile([128, 128], f16, tag=f"ucat_{h}")
            nc.scalar.dma_start(t[:], ucat_d.ap()[h])
            ucat_t.append(t)
        ident_t = const_pool.tile([64, 64], f16, tag="ident")
        nc.scalar.dma_start(ident_t[:], ident_d.ap()[:, :])

        # ---- input DMAs (sync ring, FIFO = priority order) ----
        preds_t = []
        for h in range(2):
            t = data_pool.tile([128, K * WF], f32, tag=f"preds{h}")
            preds_t.append(t)
        mask_t = data_pool.tile([128, 2 * WF], f16, tag="mask")
        ft_big = data_pool.tile([128, 32 * (C + 1)], f16, tag="ftbig")
        pvs = [preds_t[h][:].rearrange("p (k w) -> p k w", k=K)
               for h in range(2)]
        nc.sync.dma_start(pvs[0][:, 0:KA, :], preds_d.ap()[0][:, 0:KA, :])
        nc.sync.dma_start(pvs[0][:, KA:K, :], preds_d.ap()[0][:, KA:K, :])
        nc.sync.dma_start(mask_t[:], mask_d.ap())
        nc.sync.dma_start(pvs[1][:, 0:KA, :], preds_d.ap()[1][:, 0:KA, :])
        nc.sync.dma_start(pvs[1][:, KA:K, :], preds_d.ap()[1][:, KA:K, :])
        nc.sync.dma_start(ft_big[:], feats_d.ap())

        # ---- one-hot of mask: oh4[p, k, h2, wf] via tensor_scalar (4x) ----
        oh4 = data_pool.tile([128, K * 2 * WF], f16, tag="oh4")
        ohv = oh4[:].rearrange("p (k x) -> p k x", k=K)
        for k in range(K):
            nc.vector.tensor_scalar(
                ohv[:, k, :], mask_t[:], float(k), None, OP.is_equal)
        ohv4 = oh4[:].rearrange("p (k h w) -> p k h w", k=K, h=2)

        # ---- per-half f32 max over classes: TT-max trees chasing the two
        # preds DMA k-chunks; temps shared across halves ----
        t5a = data_pool.tile([128, 5 * WF], f32, tag="t5a")
        v5a = t5a[:].rearrange("p (k w) -> p k w", k=5)
        t5b = data_pool.tile([128, 5 * WF], f32, tag="t5b")
        v5b = t5b[:].rearrange("p (k w) -> p k w", k=5)
        t2 = data_pool.tile([128, 2 * WF], f32, tag="t2")
        v2 = t2[:].rearrange("p (k w) -> p k w", k=2)
        m1 = data_pool.tile([128, WF], f32, tag="m1")
        m2 = data_pool.tile([128, WF], f32, tag="m2")
        mA = data_pool.tile([128, WF], f32, tag="mA")
        maxv_t = []
        for h in range(2):
            mx = data_pool.tile([128, WF], f32, tag=f"maxv_{h}")
            maxv_t.append(mx)

        def _tree(h):
            pvh, mx = pvs[h], maxv_t[h]
            dve = nc.vector
            # chunk A: classes [0, 11) -> mA
            dve.tensor_tensor(v5a, pvh[:, 0:5, :], pvh[:, 5:10, :], op=OP.max)
            dve.tensor_tensor(v2, v5a[:, 0:2, :], v5a[:, 2:4, :], op=OP.max)
            dve.tensor_tensor(m1[:], v2[:, 0, :], v2[:, 1, :], op=OP.max)
            dve.tensor_tensor(m2[:], m1[:], v5a[:, 4, :], op=OP.max)
            dve.tensor_tensor(mA[:], m2[:], pvh[:, 10, :], op=OP.max)
            # chunk B: classes [11, 21) -> merge into maxv
            dve.tensor_tensor(v5b, pvh[:, 11:16, :], pvh[:, 16:21, :],
                              op=OP.max)
            dve.tensor_tensor(v2, v5b[:, 0:2, :], v5b[:, 2:4, :], op=OP.max)
            dve.tensor_tensor(m1[:], v2[:, 0, :], v2[:, 1, :], op=OP.max)
            dve.tensor_tensor(m2[:], m1[:], v5b[:, 4, :], op=OP.max)
            dve.tensor_tensor(mx[:], m2[:], mA[:], op=OP.max)

        eq_t = []
        joint_t = []
        for h in range(2):
            eqh = data_pool.tile([128, KW], f16, tag=f"eq{h}")
            eq_t.append(eqh)
            jh = data_pool.tile([128, KW], f16, tag=f"joint{h}")
            joint_t.append(jh)
        eqv = [eq_t[h][:].rearrange("p (k w) -> p k w", k=K) for h in range(2)]
        jv = [joint_t[h][:].rearrange("p (k w) -> p k w", k=K)
              for h in range(2)]

        # DVE stream: tree of half h, then its eq/mul granules (3 of 7
        # classes — DVE per-op overhead is ~160ns, keep ops big), then the
        # next half — h0 compute overlaps the h1 preds DMA.
        NG = 7
        for h in range(2):
            _tree(h)
            for g0 in range(0, K, NG):
                bc = (maxv_t[h][:].unsqueeze(1).to_broadcast([128, NG, WF]))
                nc.vector.tensor_tensor(
                    eqv[h][:, g0:g0 + NG, :], pvs[h][:, g0:g0 + NG, :],
                    bc, op=OP.is_ge)
                nc.vector.tensor_tensor(
                    jv[h][:, g0:g0 + NG, :], eqv[h][:, g0:g0 + NG, :],
                    ohv4[:, g0:g0 + NG, h, :], op=OP.mult)

        # ---- stage 1 per half: A_h = Uh_h^T @ joint_h (no cross-half
        # accumulation, so the whole h0 chain runs while the DVE is busy
        # with h1); transpose each class pair into at_h[wf, k, v, hc] ----
        a_t = []
        at_big = []
        atv = []
        for h in range(2):
            ah = data_pool.tile([64, KW], f16, tag=f"a{h}")
            a_t.append(ah)
            ath = data_pool.tile([128, K * 2 * HC], f16, tag=f"at{h}")
            at_big.append(ath)
            atv.append(ath[:].rearrange("p (k v h) -> p k v h", k=K, v=2))

        n_kc = (K + 1) // 2  # 11 chunks of 2 classes (last has 1)
        for h in range(2):
            for kc in range(n_kc):
                k0 = 2 * kc
                nk = min(2, K - k0)
                w = nk * WF
                fc = k0 * WF
                ps = ps1_pool.tile([64, 512], f32, tag="ps1")
                nc.tensor.matmul(ps[:, :w], u16_t[h][:, :],
                                 joint_t[h][:, fc:fc + w],
                                 start=True, stop=True)
                nc.scalar.copy(a_t[h][:, fc:fc + w], ps[:, :w])
                pst = pst_pool.tile([128, 256], f16, tag="pst")
                for u in range(2 * nk):
                    k = k0 + u // 2
                    wh = u % 2
                    nc.tensor.transpose(
                        pst[:, u * 64:(u + 1) * 64],
                        a_t[h][:, k * WF + wh * 128: k * WF + wh * 128 + 128],
                        ident_t[:])
                nc.scalar.copy(
                    atv[h][:, k0:k0 + nk, :, :],
                    pst[:, :nk * 128].rearrange("p (n v h) -> p n v h",
                                                n=nk, v=2))

        # ---- stage 2: ds on 128 partitions via [U|U] stationary; each
        # psb chunk accumulates 4 contributions (half x wf-half). psb is
        # (k, hc) so the moving runs are contiguous; b_sh is (hc, k) with
        # odd hc shifted into the upper partitions (free-dim shift in the
        # copies; the hi copy runs on the then-idle DVE). ----
        b_sh = data_pool.tile([128, HC * K], f16, tag="bsh")
        bshv = b_sh[:].rearrange("p (h k) -> p h k", h=HC)
        atm = [at_big[h][:].rearrange("p (k v h) -> p v k h", k=K, v=2)
               for h in range(2)]
        for c in range(4):
            h0 = 16 * c
            nh = 17 if c < 3 else 16  # one-row overlap feeds the odd shift
            psb = psb_pool.tile([128, K * 17], f32, tag="psb")
            pbv = psb[:].rearrange("p (k h) -> p k h", h=17)
            for h in range(2):
                for v in range(2):
                    nc.tensor.matmul(pbv[:, :, :nh], ucat_t[v][:, :],
                                     atm[h][:, v, :, h0:h0 + nh],
                                     start=(h == 0 and v == 0),
                                     stop=(h == 1 and v == 1))
            # transposed-view copies: psb (k, h) -> b_sh (h, k)
            nc.scalar.copy(bshv[0:64, h0:h0 + 16, :],
                           pbv[0:64, :, 0:16].transpose([0, 2, 1]))
            nhi = 16 if c < 3 else 15
            nc.vector.tensor_copy(bshv[64:128, h0:h0 + nhi, :],
                                  pbv[64:128, :, 1:1 + nhi]
                                  .transpose([0, 2, 1]))

        # ---- final: out[k, c] = sum_q ds[q, k] feats^T[q, c]; the 257th
        # feats column is 1.0 so column C lands counts[k] for free ----
        ftv = ft_big[:].rearrange("p (x c) -> p x c", x=32)
        psf = psf_pool.tile([K, C + 1], f32, tag="fin")
        for ch in range(32):
            nc.tensor.matmul(
                psf[:, :],
                bshv[:, 2 * ch, :],
                ftv[:, ch, :],
                start=(ch == 0), stop=(ch == 31))
        res_t = res_pool.tile([K, C + 1], f32, tag="res")
        nc.scalar.copy(res_t[:], psf[:])
        nc.sync.dma_start(out_d.ap()[:, :], res_t[:])

    nc.compile()
    return nc


def _get_program():
    if "nc" not in _PROGRAM_CACHE:
        _PROGRAM_CACHE["nc"] = _build_program()
    return _PROGRAM_CACHE["nc"]


def _host_inputs(feats, preds, masks):
    U = _upsample_matrix(HC, HF)  # [256, 64] f32, entries multiples of 1/8
    u16 = U.reshape(2, 128, HC).astype(np.float16)
    ucat = np.concatenate([u16, u16], axis=2)  # [2, 128, 128]
    ident = np.eye(64, dtype=np.float16)

    # layouts: every device tensor is contiguous per SBUF partition
    preds32 = np.asarray(preds, dtype=np.float32)  # [B, K, 256, 256]
    predsx = np.ascontiguousarray(
        preds32.reshape(B, K, 2, 128, WF).transpose(0, 2, 3, 1, 4))
    mask16 = np.ascontiguousarray(
        np.asarray(masks).astype(np.float16)
        .reshape(B, 2, 128, WF).transpose(0, 2, 1, 3))
    feats32 = np.asarray(feats, dtype=np.float32).reshape(B, C, PIX)

    in_maps = []
    for b in range(B):
        ft = np.empty((PIX, C + 1), dtype=np.float16)
        ft[:, :C] = feats32[b].T
        ft[:, C] = 1.0  # ones column -> counts fall out of the final matmul
        ftx = np.ascontiguousarray(
            ft.reshape(32, 128, C + 1).transpose(1, 0, 2))
        in_maps.append({
            "preds": predsx[b],
            "mask": mask16[b],
            "feats": ftx,
            "u": u16,
            "ucat": ucat,
            "ident": ident,
        })
    return in_maps


def kernel(feats, preds, masks, _results_hook=None):
    from concourse.bass_utils import run_bass_kernel_spmd

    nc = _get_program()
    in_maps = _host_inputs(feats, preds, masks)
    res = run_bass_kernel_spmd(nc, in_maps, list(range(N_CORES)))
    if _results_hook is not None:
        _results_hook(res)

    protos = []
    for b in range(B):
        out = res.results[b]["out"]  # [K, C+1] f32
        sums = out[:, :C]            # [K, C]
        counts = out[:, C]           # [K]
        protos.append(sums / (counts + EPS)[:, None])  # [K, C]
    return np.mean(np.stack(protos), axis=0).astype(np.float32)
